# revision 7
# baseline (speedup 1.0000x reference)
"""Trainium2 Bass kernel for multi-head attention (B=4, N=2048, C=768, H=12).

Sharding: 8 cores = 4 batches x 2 head-halves. Each core computes Q/K/V and
attention for its 6 heads (3 head-pairs) over the full 2048-token sequence,
then the final projection restricted to its 384 feature columns, producing a
partial [2048, 768] output. The host sums the two partials per batch (the
even core folds in the bias). No duplicated projection work, no collectives.

All matmul operands are fp16 (1 cyc/row on the PE; fp32 runs a 2x-slower
2-pass HIGH mode), accumulation is f32 in PSUM, softmax normalization in f32.

Per pair p: K/Q proj for p (and V for all pairs, once) write persistent SBUF
tiles; the Tile scheduler overlaps pair p+1's projections with pair p's
attention, which is paced by ScalarE exp (the hard floor: ~25M exps/core).
Attention per (pair, 512-query block ib, 128-key tile jt):
  ss[j, i]   = kT_h.T @ qT_h     (2 heads row-tiled, concurrent on the PE)
  et         = exp(SCALE*ss)     (ScalarE, PSUM->SBUF fp16, scale folded)
  po[0:65,i] += v'[j,0:65].T @ et  (v' = [v_h | ones]; row 64 = softmax denom)
normalize: outT = po[0:64] * bcast(1/po[64]); a pair's two denominators are
stacked at partitions 0/32 of one tile via tiny SBUF DMAs, one DVE reciprocal
covers both, GpSimd partition_broadcast replicates (source must be partition
0) and DVE multiplies. The final projection for query block ib runs right
after the last pair's normalization of ib, hiding under the remaining
attention stream; only the last block's projection is a tail.
"""

import numpy as np

B, N, C = 4, 2048, 768
H, HD = 12, 64
SCALE = HD ** -0.5
P = 128
CT = C // P          # 6 contraction tiles for QKV projections
HC = C // 2          # 384 feature columns per core
PCT = HC // P        # 3 contraction tiles for the final projection
PAIRS = 3            # head pairs per core
JT = N // P          # 16 key tiles
IB = N // 512        # 4 query blocks
TKB = 512            # token-block width of projection matmuls
NCORES = 8

_cache = {}


def _build_bass():
    import concourse.bass as bass
    import concourse.tile as tile
    import concourse.mybir as mybir
    from concourse import bacc
    from concourse.bass import ts, ds
    from contextlib import ExitStack

    fr = mybir.dt.float32r
    f32 = mybir.dt.float32
    f16 = mybir.dt.float16
    Exp = mybir.ActivationFunctionType.Exp

    nc = bacc.Bacc("TRN2", target_bir_lowering=False, debug=False)

    xt_d = nc.dram_tensor("xt", [C, N], f16, kind="ExternalInput").ap()
    wq_d = nc.dram_tensor("wq", [C, HC], f16, kind="ExternalInput").ap()
    wk_d = nc.dram_tensor("wk", [C, HC], f16, kind="ExternalInput").ap()
    wv_d = nc.dram_tensor("wv", [C, HC], f16, kind="ExternalInput").ap()
    wp_d = nc.dram_tensor("wp", [HC, C], f16, kind="ExternalInput").ap()
    bb_d = nc.dram_tensor("bb", [P, C], f32, kind="ExternalInput").ap()
    out_d = nc.dram_tensor("out", [N, C], f32, kind="ExternalOutput").ap()

    xt_r = xt_d.rearrange("(o p) n -> p o n", p=P)
    wq_r = wq_d.rearrange("(o p) n -> p o n", p=P)
    wk_r = wk_d.rearrange("(o p) n -> p o n", p=P)
    wv_r = wv_d.rearrange("(o p) n -> p o n", p=P)
    wp_r = wp_d.rearrange("(o p) n -> p o n", p=P)
    out_r = out_d.rearrange("(t p) n -> t p n", p=P)

    with tile.TileContext(nc) as tc:
        with ExitStack() as ctx:
            persist = ctx.enter_context(tc.tile_pool(name="persist", bufs=1))
            # full x kept resident: [128, 6 ctiles, 2048 tokens] fp16
            xt_sb = persist.tile([P, CT, N], f16, name="xt_sb")
            nc.sync.dma_start(xt_sb[:], xt_r)
            wq_sb = persist.tile([P, CT, HC], f16, name="wq_sb")
            nc.sync.dma_start(wq_sb[:], wq_r)
            wk_sb = persist.tile([P, CT, HC], f16, name="wk_sb")
            nc.sync.dma_start(wk_sb[:], wk_r)
            wv_sb = persist.tile([P, CT, HC], f16, name="wv_sb")
            nc.sync.dma_start(wv_sb[:], wv_r)
            wp_sb = persist.tile([P, PCT, C], f16, name="wp_sb")
            nc.sync.dma_start(wp_sb[:], wp_r)
            bias_sb = persist.tile([P, C], f32, name="bias_sb")
            nc.sync.dma_start(bias_sb[:], bb_d)

            # pair-packed K/Q: partitions 0:64 even head, 64:128 odd head
            kT_sb = persist.tile([P, PAIRS, N], f16, name="kT_sb")
            qT_sb = persist.tile([P, PAIRS, N], f16, name="qT_sb")
            # V + ones column: [keys 128, key-tile, head, 66] (col 64 = ones)
            v_all = persist.tile([P, JT, 6, 66], f16, name="v_all")
            outT_sb = persist.tile([P, PAIRS, N], f16, name="outT_sb")
            with nc.allow_low_precision(reason="ones constant is exact in f16"):
                nc.vector.tensor_copy(
                    v_all[:, :, :, 64], nc.const_aps.tensor(1.0, [P, JT, 6], f32)
                )

            apsum = ctx.enter_context(
                tc.tile_pool(name="apsum", bufs=2, space="PSUM")
            )
            spsum = ctx.enter_context(
                tc.tile_pool(name="spsum", bufs=2, space="PSUM")
            )
            opsum = ctx.enter_context(
                tc.tile_pool(name="opsum", bufs=2, space="PSUM")
            )
            expt_pool = ctx.enter_context(tc.tile_pool(name="expt", bufs=4))
            nrm_pool = ctx.enter_context(tc.tile_pool(name="nrm", bufs=2))
            poS_pool = ctx.enter_context(tc.tile_pool(name="poSp", bufs=4))
            outsb_pool = ctx.enter_context(tc.tile_pool(name="outsb", bufs=2))

            def kq_proj(p):
                for tb in range(N // TKB):
                    for w_sb, dst in ((wk_sb, kT_sb), (wq_sb, qT_sb)):
                        ps = apsum.tile([P, TKB], f32, tag="aps")
                        for c in range(CT):
                            nc.tensor.matmul(
                                ps[:],
                                w_sb[:, c, ts(p, P)],
                                xt_sb[:, c, ts(tb, TKB)],
                                start=(c == 0),
                                stop=(c == CT - 1),
                            )
                        with nc.allow_low_precision(reason="f16 kq path"):
                            nc.vector.tensor_copy(dst[:, p, ts(tb, TKB)], ps[:])

            def v_proj():
                # all 6 heads at once: token-tile stationary, wv moving
                for tt in range(JT):
                    ps = apsum.tile([P, TKB], f32, tag="aps")
                    for c in range(CT):
                        nc.tensor.matmul(
                            ps[:, 0:HC],
                            xt_sb[:, c, ts(tt, P)],
                            wv_sb[:, c, :],
                            start=(c == 0),
                            stop=(c == CT - 1),
                        )
                    with nc.allow_low_precision(reason="f16 value path"):
                        nc.vector.tensor_copy(
                            v_all[:, tt, :, 0:64],
                            ps[:, 0:HC].rearrange("p (h e) -> p h e", e=64),
                        )

            def out_proj(ib):
                # final projection for the 4 token tiles of query block ib
                for g in range(4):
                    git = 4 * ib + g
                    ob = outsb_pool.tile([P, C], f32, tag="ob")
                    for n0, n1 in ((0, 512), (512, 768)):
                        pp = apsum.tile([P, TKB], f32, tag="aps")
                        for t in range(PAIRS):
                            nc.tensor.matmul(
                                pp[:, 0 : n1 - n0],
                                outT_sb[:, t, ds(git * P, P)],
                                wp_sb[:, t, n0:n1],
                                start=(t == 0),
                                stop=(t == PAIRS - 1),
                            )
                        nc.vector.tensor_add(
                            ob[:, n0:n1], pp[:, 0 : n1 - n0], bias_sb[:, n0:n1]
                        )
                    nc.sync.dma_start(out_r[git], ob[:])

            def attention(p, ib):
                den_q = nrm_pool.tile([33, 512], fr, tag="den_q")
                with nc.allow_low_precision(reason="f32r is bitwise f32"):
                    nc.vector.tensor_copy(
                        den_q[:], nc.const_aps.tensor(1.0, [33, 512], f32)
                    )
                po0 = opsum.tile([P, 512], f32, tag="po")
                po1 = opsum.tile([P, 512], f32, tag="po")
                pos = (po0, po1)
                for jt in range(JT):
                    ss = spsum.tile([P, 1024], f32, tag="ss")
                    nc.tensor.matmul(
                        ss[:, 0:512],
                        kT_sb[0:64, p, ts(jt, P)],
                        qT_sb[0:64, p, ts(ib, 512)],
                        start=True,
                        stop=True,
                    )
                    nc.tensor.matmul(
                        ss[:, 512:1024],
                        kT_sb[64:128, p, ts(jt, P)],
                        qT_sb[64:128, p, ts(ib, 512)],
                        start=True,
                        stop=True,
                    )
                    et = expt_pool.tile([P, 1024], f16, tag="et")
                    nc.scalar.activation(et[:], ss[:], Exp, scale=SCALE)
                    for hh in range(2):
                        nc.tensor.matmul(
                            pos[hh][0:65, :],
                            v_all[:, jt, 2 * p + hh, 0:65],
                            et[:, hh * 512 : (hh + 1) * 512],
                            start=(jt == 0),
                            stop=(jt == JT - 1),
                        )
                poS_all = []
                for hh in range(2):
                    poS = poS_pool.tile([65, 512], fr, tag="poS")
                    with nc.allow_low_precision(reason="f32r is bitwise f32"):
                        nc.vector.tensor_copy(poS[:], pos[hh][0:65, :])
                    # stack this head's denominator at partition 32*hh
                    nc.sync.dma_start(
                        den_q[32 * hh : 32 * hh + 1, :], poS[64:65, :]
                    )
                    poS_all.append(poS)
                rd_q = nrm_pool.tile([33, 512], fr, tag="rd_q")
                with nc.allow_low_precision(reason="f32r is bitwise f32"):
                    nc.vector.reciprocal(rd_q[:], den_q[:])
                for hh in range(2):
                    if hh == 0:
                        rd_src = rd_q
                    else:
                        # relocate head-1's reciprocal to partition 0:
                        # HW partition_broadcast only sources partition 0
                        rd_src = nrm_pool.tile([1, 512], fr, tag="rd1", name="rd1")
                        nc.sync.dma_start(rd_src[:], rd_q[32:33, :])
                    rb_sb = nrm_pool.tile([64, 512], fr, tag="rb_sb")
                    nc.gpsimd.partition_broadcast(rb_sb[:], rd_src[0:1, :])
                    with nc.allow_low_precision(reason="f16 attn output"):
                        nc.vector.tensor_mul(
                            outT_sb[
                                hh * 64 : (hh + 1) * 64, p, ts(ib, 512)
                            ],
                            poS_all[hh][0:64, :],
                            rb_sb[:],
                        )

            kq_proj(0)
            v_proj()
            for p in range(PAIRS):
                if p + 1 < PAIRS:
                    kq_proj(p + 1)
                for ib in range(IB):
                    attention(p, ib)
                    if p == PAIRS - 1:
                        out_proj(ib)

    nc.compile()
    return nc


def _get_nc():
    if "nc" not in _cache:
        _cache["nc"] = _build_bass()
    return _cache["nc"]


def _prep_in_maps(x, w_qkv, w_proj, b_proj):
    x = np.asarray(x, np.float32)
    w_qkv = np.asarray(w_qkv, np.float32)
    w_proj = np.asarray(w_proj, np.float32)
    b_proj = np.asarray(b_proj, np.float32)

    wq = np.ascontiguousarray(w_qkv[0:C].T).astype(np.float16)
    wk = np.ascontiguousarray(w_qkv[C : 2 * C].T).astype(np.float16)
    wv = np.ascontiguousarray(w_qkv[2 * C : 3 * C].T).astype(np.float16)
    wp = np.ascontiguousarray(w_proj.T).astype(np.float16)
    bb = np.ascontiguousarray(np.broadcast_to(b_proj[None, :], (P, C)))
    zb = np.zeros((P, C), np.float32)

    in_maps = []
    for core in range(NCORES):
        b, half = core // 2, core % 2
        xt = np.ascontiguousarray(x[b].T).astype(np.float16)
        sl = slice(half * HC, (half + 1) * HC)
        in_maps.append(
            {
                "xt": xt,
                "wq": np.ascontiguousarray(wq[:, sl]),
                "wk": np.ascontiguousarray(wk[:, sl]),
                "wv": np.ascontiguousarray(wv[:, sl]),
                "wp": np.ascontiguousarray(wp[sl, :]),
                "bb": bb if half == 0 else zb,
            }
        )
    return in_maps


def run(x, w_qkv, w_proj, b_proj, trace=False):
    from concourse import bass_utils

    nc = _get_nc()
    in_maps = _prep_in_maps(x, w_qkv, w_proj, b_proj)
    br = bass_utils.run_bass_kernel_spmd(
        nc, in_maps, core_ids=list(range(NCORES)), trace=trace
    )
    y = np.empty((B, N, C), np.float32)
    for b in range(B):
        y[b] = br.results[2 * b]["out"]
        y[b] += br.results[2 * b + 1]["out"]
    return y, br


def kernel(x, w_qkv, w_proj, b_proj):
    y, _ = run(x, w_qkv, w_proj, b_proj, trace=False)
    return y
